# revision 8
# baseline (speedup 1.0000x reference)
"""Trainium2 Bass kernel for nn_Block_82592221102594 (moe_routing).

Contract: kernel(**inputs) takes FULL unsharded inputs (keyed as in
setup_inputs()) and returns the FULL output (x2, gating_loss).

Sharding: data-parallel over batch B=8 across the 8 NeuronCores (one
batch element per core). Attention, LayerNorms and MoE are all
batch-element independent; the gating aux loss needs global means over
tokens, which the host computes from the per-core router probabilities
returned by the device.
"""

import sys

for _p in ("/opt/trn_rl_repo",):
    if _p not in sys.path:
        sys.path.insert(0, _p)

import numpy as np
import ml_dtypes

L, B, E, H, NE, TOPK = 1024, 8, 512, 8, 8, 2
HD = E // H          # 64
P = 128              # partitions
KT = E // P          # 4 k-tiles of 128 over the feature dim
TT = L // P          # 8 token tiles of 128
N_CORES = 8
SLOPE = 0.01
EPS = 1e-5

_CACHE: dict = {}

bf16 = ml_dtypes.bfloat16


def _register_leaky_w_acc():
    """Custom fused DVE op: out = leaky_relu(in0) * s0 + in1.

    body = max(Src0 * C2, Src0) * C0 + Src1  with C2 = slope (imm2),
    C0 = s0 (per-partition combine weight), Src1 = accumulator.
    Returns the DveOp, or None if registration fails (caller falls back).
    """
    import concourse.dve_ops as dve_ops
    from concourse.dve_spec import Spec, Src0, Src1, C0, C2, maxx, lower, _has_src1
    from concourse.dve_uop import DveOpSpec

    name = "LEAKY_W_ACC_ANT"
    if any(op.name == name for op in dve_ops.OPS):
        return next(op for op in dve_ops.OPS if op.name == name)

    spec = Spec(
        body=maxx(Src0 * C2, Src0) * C0 + Src1,
        reference=lambda in0, in1, s0, s1, imm2: (
            np.maximum(in0 * imm2, in0) * s0 + in1
        ),
    )
    row = dve_ops._CUSTOM_DVE_ROW_BASE + len(dve_ops.OPS)
    assert row < 0x20
    shas = {}
    for ver in ("v3", "v4"):
        try:
            s = DveOpSpec(
                name=name, opcode=row, uops=lower(spec, ver=ver),
                rd1_en=_has_src1(spec),
            ).sha(ver)
            shas[ver] = s
        except Exception:
            pass
    if not shas:
        return None
    op = dve_ops.DveOp(name, spec, subdim=False, uops_sha=shas)
    dve_ops.OPS.append(op)
    dve_ops.CUSTOM_DVE_SPECS[name] = spec
    dve_ops._SUB_OPCODE_FOR_NAME[name] = row
    return op


def _build(nonzero_bias: bool, ln1_affine: bool, ln2_affine: bool):
    """Build the SPMD single-core Bass program. Returns (nc, names)."""
    import concourse.bass as bass
    import concourse.mybir as mybir
    import concourse.tile as tile
    from concourse import bacc
    from concourse.masks import make_identity

    dt = mybir.dt
    f32 = dt.float32
    b16 = dt.bfloat16
    ALU = mybir.AluOpType
    ACT = mybir.ActivationFunctionType

    leaky_op = _register_leaky_w_acc()

    nc = bacc.Bacc("TRN2", target_bir_lowering=False, debug=False,
                   num_devices=N_CORES)

    # ---- DRAM I/O ----------------------------------------------------
    xT_d = nc.dram_tensor("xT", [E, L], b16, kind="ExternalInput")
    xtok_d = nc.dram_tensor("xtok", [L, E], f32, kind="ExternalInput")
    wqT_d = nc.dram_tensor("wqT", [E, E], b16, kind="ExternalInput")
    wkT_d = nc.dram_tensor("wkT", [E, E], b16, kind="ExternalInput")
    wvT_d = nc.dram_tensor("wvT", [E, E], b16, kind="ExternalInput")
    owT_d = nc.dram_tensor("owT", [E, E], b16, kind="ExternalInput")
    shT_d = nc.dram_tensor("shT", [E, E], b16, kind="ExternalInput")
    expT_d = nc.dram_tensor("expT", [NE, E, E], b16, kind="ExternalInput")
    gwT_d = nc.dram_tensor("gwT", [E, NE], b16, kind="ExternalInput")
    if nonzero_bias:
        # rows: bq_eff, bk_eff, bv_eff, out_b, sh_b, gate_b(padded), exp_b x8
        bias_d = nc.dram_tensor("biases", [13, E], b16, kind="ExternalInput")
    if ln1_affine or ln2_affine:
        lnw_d = nc.dram_tensor("lnw", [4, E], f32, kind="ExternalInput")

    x2_d = nc.dram_tensor("x2", [L, E], f32, kind="ExternalOutput")
    probs_d = nc.dram_tensor("probs", [L, NE], f32, kind="ExternalOutput")

    with tile.TileContext(nc) as tc:
        import contextlib
        est = contextlib.ExitStack()
        with est:
            const = est.enter_context(tc.tile_pool(name="const", bufs=1))
            wpool = est.enter_context(tc.tile_pool(name="weights", bufs=1))
            apool = est.enter_context(tc.tile_pool(name="acts", bufs=1))

            ident = const.tile([P, P], b16)
            make_identity(nc, ident)
            eps_t = const.tile([P, 1], f32)
            nc.vector.memset(eps_t, EPS)

            # ---- weight / input DMAs ---------------------------------
            xT_sb = apool.tile([P, KT, L], b16)      # x^T feature-major
            nc.sync.dma_start(out=xT_sb, in_=xT_d.ap().rearrange(
                "(a p) n -> p a n", p=P))
            xtok_sb = apool.tile([P, TT, E], f32)    # x token-major (residual)
            nc.sync.dma_start(out=xtok_sb, in_=xtok_d.ap().rearrange(
                "(t p) e -> p t e", p=P))

            def load_w(dram):
                t = wpool.tile([P, KT, E], b16)
                nc.sync.dma_start(out=t, in_=dram.ap().rearrange(
                    "(a p) o -> p a o", p=P))
                return t

            wqT_sb = load_w(wqT_d)
            wkT_sb = load_w(wkT_d)
            wvT_sb = load_w(wvT_d)
            owT_sb = load_w(owT_d)
            shT_sb = load_w(shT_d)
            expT_sb = wpool.tile([P, NE, KT, E], b16)
            nc.sync.dma_start(out=expT_sb, in_=expT_d.ap().rearrange(
                "n (a p) o -> p n a o", p=P))
            gwT_sb = wpool.tile([P, KT, NE], b16)
            nc.sync.dma_start(out=gwT_sb, in_=gwT_d.ap().rearrange(
                "(a p) o -> p a o", p=P))
            if nonzero_bias:
                bias_sb = const.tile([1, 13, E], b16)
                nc.sync.dma_start(out=bias_sb, in_=bias_d.ap()[None, :, :])
                ones_row = const.tile([1, L], b16)
                nc.vector.memset(ones_row, 1.0)
            if ln1_affine or ln2_affine:
                # pre-broadcast LN gains/biases to 128 partitions via DMA
                lnw_sb = const.tile([P, 4, E], f32)
                nc.sync.dma_start(
                    out=lnw_sb,
                    in_=bass.AP(tensor=lnw_d, offset=0,
                                ap=[[0, P]] + lnw_d.ap().ap))

            # ---- phase 1: QKV projections ----------------------------
            qT_sb = apool.tile([P, KT, L], b16)
            kT_sb = apool.tile([P, KT, L], b16)
            v_sb = apool.tile([P, TT, H, HD + 1], b16)
            nc.vector.memset(v_sb[:, :, :, HD:HD + 1], 1.0)

            with tc.tile_pool(name="p1qk", bufs=2, space="PSUM") as pqk, \
                 tc.tile_pool(name="p1v", bufs=2, space="PSUM") as pv:
                # q^T, k^T feature-major: out[o, t] = sum_k W^T[k,o] x^T[k,t]
                for dst, w_sb, brow in ((qT_sb, wqT_sb, 0), (kT_sb, wkT_sb, 1)):
                    for o in range(KT):
                        ps = pqk.tile([P, L], f32, tag="qk")
                        for c in range(2):
                            for k in range(KT):
                                nc.tensor.matmul(
                                    ps[:, c * 512:(c + 1) * 512],
                                    w_sb[:, k, o * P:(o + 1) * P],
                                    xT_sb[:, k, c * 512:(c + 1) * 512],
                                    start=(k == 0), stop=(k == KT - 1 and
                                                          not nonzero_bias))
                            if nonzero_bias:
                                nc.tensor.matmul(
                                    ps[:, c * 512:(c + 1) * 512],
                                    bias_sb[:, brow, o * P:(o + 1) * P],
                                    ones_row[:, c * 512:(c + 1) * 512],
                                    start=False, stop=True)
                        nc.vector.tensor_copy(out=dst[:, o, :], in_=ps)
                # v token-major: out[t, o] = sum_k x^T[k,t] WvT[k,o]
                for t in range(TT):
                    ps = pv.tile([P, E], f32, tag="v")
                    for k in range(KT):
                        nc.tensor.matmul(
                            ps, xT_sb[:, k, t * P:(t + 1) * P],
                            wvT_sb[:, k, :],
                            start=(k == 0), stop=(k == KT - 1 and
                                                  not nonzero_bias))
                    if nonzero_bias:
                        nc.tensor.matmul(ps, ones_row[:, t * P:(t + 1) * P],
                                         bias_sb[:, 2, :], start=False,
                                         stop=True)
                    nc.vector.tensor_copy(
                        out=v_sb[:, t, :, 0:HD],
                        in_=ps.rearrange("p (h d) -> p h d", h=H))

            # ---- phase 2: attention ----------------------------------
            aoT_sb = apool.tile([P, KT, L], b16)
            scale = 1.0 / float(np.sqrt(HD))
            with tc.tile_pool(name="p2s", bufs=2, space="PSUM") as psS, \
                 tc.tile_pool(name="p2a", bufs=2, space="PSUM") as psA, \
                 tc.tile_pool(name="p2e", bufs=10) as peE, \
                 tc.tile_pool(name="p2n", bufs=3) as pnorm:
                for h in range(H):
                    kt_i, kt_off = h // 2, (h % 2) * HD
                    kh = kT_sb[kt_off:kt_off + HD, kt_i, :]   # [64, L]
                    qh = qT_sb[kt_off:kt_off + HD, kt_i, :]   # [64, L]
                    e_tiles = []
                    for j in range(TT):
                        sp = psS.tile([P, L], f32, tag="S")
                        for c in range(2):
                            nc.tensor.matmul(
                                sp[:, c * 512:(c + 1) * 512],
                                kh[:, j * P:(j + 1) * P],
                                qh[:, c * 512:(c + 1) * 512],
                                start=True, stop=True)
                        et = peE.tile([P, L], b16, tag="E")
                        nc.scalar.activation(out=et, in_=sp, func=ACT.Exp,
                                             scale=scale)
                        e_tiles.append(et)
                    ap = psA.tile([HD + 1, L], f32, tag="A")
                    for j in range(TT):
                        for c in range(2):
                            nc.tensor.matmul(
                                ap[:, c * 512:(c + 1) * 512],
                                v_sb[:, j, h, :],
                                e_tiles[j][:, c * 512:(c + 1) * 512],
                                start=(j == 0), stop=(j == TT - 1))
                    rec = pnorm.tile([1, L], f32, tag="rec")
                    nc.vector.reciprocal(rec, ap[HD:HD + 1, :])
                    rb = pnorm.tile([HD, L], f32, tag="rb")
                    nc.gpsimd.partition_broadcast(rb, rec)
                    # 64-wide DVE op: reads (psum base 0) can come from any
                    # partitions; the write window is parts 0-63 or 64-127.
                    nc.vector.tensor_tensor(
                        out=aoT_sb[kt_off:kt_off + HD, kt_i, :],
                        in0=ap[0:HD, :], in1=rb, op=ALU.mult)

            # ---- phase 3: out_proj + residual + LN1 ------------------
            x1_sb = apool.tile([P, TT, E], f32)       # x1 token-major fp32
            x1n_sb = apool.tile([P, TT, E], b16)      # normalized x1 (bf16)
            x1T_sb = apool.tile([P, KT, L], b16)      # x1n^T feature-major
            mv1 = apool.tile([P, TT, 2], f32)
            rstd1 = apool.tile([P, TT], f32)
            stat_pool = est.enter_context(tc.tile_pool(name="stats", bufs=4))

            with tc.tile_pool(name="p3o", bufs=2, space="PSUM") as po:
                for t in range(TT):
                    ps = po.tile([P, E], f32, tag="o")
                    for k in range(KT):
                        nc.tensor.matmul(
                            ps, aoT_sb[:, k, t * P:(t + 1) * P],
                            owT_sb[:, k, :],
                            start=(k == 0), stop=(k == KT - 1 and
                                                  not nonzero_bias))
                    if nonzero_bias:
                        nc.tensor.matmul(ps, ones_row[:, t * P:(t + 1) * P],
                                         bias_sb[:, 3, :], start=False,
                                         stop=True)
                    # x1 = x + attn_out
                    nc.vector.scalar_tensor_tensor(
                        out=x1_sb[:, t, :], in0=ps, scalar=1.0,
                        in1=xtok_sb[:, t, :], op0=ALU.mult, op1=ALU.add)
                    st = stat_pool.tile([P, 6], f32, tag="bn")
                    nc.vector.bn_stats(out=st, in_=x1_sb[:, t, :])
                    nc.vector.bn_aggr(out=mv1[:, t, :], in_=st)
                # batched rstd for all 8 token tiles (one table-switch)
                std1 = stat_pool.tile([P, TT], f32, tag="std")
                nc.scalar.activation(out=std1, in_=mv1[:, :, 1],
                                     func=ACT.Sqrt, bias=eps_t)
                nc.vector.reciprocal(rstd1, std1)
                for t in range(TT):
                    nc.vector.tensor_scalar(
                        out=x1n_sb[:, t, :], in0=x1_sb[:, t, :],
                        scalar1=mv1[:, t, 0:1], scalar2=rstd1[:, t:t + 1],
                        op0=ALU.subtract, op1=ALU.mult)
                    if ln1_affine:
                        nc.vector.tensor_tensor(
                            out=x1n_sb[:, t, :], in0=x1n_sb[:, t, :],
                            in1=lnw_sb[:, 0, :], op=ALU.mult)
                        nc.vector.tensor_tensor(
                            out=x1n_sb[:, t, :], in0=x1n_sb[:, t, :],
                            in1=lnw_sb[:, 1, :], op=ALU.add)

            # transpose x1n -> x1T (feature-major) via PE
            with tc.tile_pool(name="p3t", bufs=2, space="PSUM") as pt:
                for t in range(TT):
                    ps = pt.tile([P, KT, P], b16, tag="tr")
                    for k in range(KT):
                        nc.tensor.transpose(ps[:, k, :],
                                            x1n_sb[:, t, k * P:(k + 1) * P],
                                            ident)
                    nc.vector.tensor_copy(out=x1T_sb[:, :, t * P:(t + 1) * P],
                                          in_=ps)

            # ---- phase 4: gate + top-2 routing -----------------------
            g_all = apool.tile([P, TT, NE], f32)
            wts = apool.tile([P, TT, NE], f32)
            probs_sb = apool.tile([P, TT, NE], f32)
            rpool = est.enter_context(tc.tile_pool(name="routing", bufs=2))
            with tc.tile_pool(name="p4g", bufs=2, space="PSUM") as pg:
                for t in range(TT):
                    ps = pg.tile([P, NE], f32, tag="g")
                    for k in range(KT):
                        nc.tensor.matmul(
                            ps, x1T_sb[:, k, t * P:(t + 1) * P],
                            gwT_sb[:, k, :],
                            start=(k == 0), stop=(k == KT - 1 and
                                                  not nonzero_bias))
                    if nonzero_bias:
                        nc.tensor.matmul(ps, ones_row[:, t * P:(t + 1) * P],
                                         bias_sb[:, 5, 0:NE], start=False,
                                         stop=True)
                    nc.vector.tensor_copy(out=g_all[:, t, :], in_=ps)
            e_all = rpool.tile([P, TT, NE], f32, tag="r0")
            nc.scalar.activation(out=e_all, in_=g_all, func=ACT.Exp)
            ssum = rpool.tile([P, TT], f32, tag="r1")
            nc.vector.reduce_sum(ssum, e_all, axis=mybir.AxisListType.X)
            rsum = rpool.tile([P, TT], f32, tag="r2")
            nc.vector.reciprocal(rsum, ssum)
            nc.vector.tensor_tensor(
                out=probs_sb, in0=e_all,
                in1=rsum[:, :, None].to_broadcast([P, TT, NE]), op=ALU.mult)
            nc.sync.dma_start(
                out=probs_d.ap().rearrange("(t p) n -> p t n", p=P),
                in_=probs_sb)
            m1 = rpool.tile([P, TT], f32, tag="r3")
            nc.vector.reduce_max(m1, probs_sb, axis=mybir.AxisListType.X)
            mask1 = rpool.tile([P, TT, NE], f32, tag="r4")
            nc.vector.tensor_tensor(
                out=mask1, in0=probs_sb,
                in1=m1[:, :, None].to_broadcast([P, TT, NE]), op=ALU.is_ge)
            pmo = rpool.tile([P, TT, NE], f32, tag="r5")
            nc.vector.tensor_tensor(out=pmo, in0=probs_sb, in1=mask1,
                                    op=ALU.mult)
            pm = rpool.tile([P, TT, NE], f32, tag="r6")
            nc.vector.tensor_tensor(out=pm, in0=probs_sb, in1=pmo,
                                    op=ALU.subtract)
            m2 = rpool.tile([P, TT], f32, tag="r7")
            nc.vector.reduce_max(m2, pm, axis=mybir.AxisListType.X)
            mask = rpool.tile([P, TT, NE], f32, tag="r8")
            nc.vector.tensor_tensor(
                out=mask, in0=probs_sb,
                in1=m2[:, :, None].to_broadcast([P, TT, NE]), op=ALU.is_ge)
            wtsu = rpool.tile([P, TT, NE], f32, tag="r9")
            nc.vector.tensor_tensor(out=wtsu, in0=probs_sb, in1=mask,
                                    op=ALU.mult)
            s2 = rpool.tile([P, TT], f32, tag="r10")
            nc.vector.reduce_sum(s2, wtsu, axis=mybir.AxisListType.X)
            r2 = rpool.tile([P, TT], f32, tag="r11")
            nc.vector.reciprocal(r2, s2)
            nc.vector.tensor_tensor(
                out=wts, in0=wtsu,
                in1=r2[:, :, None].to_broadcast([P, TT, NE]), op=ALU.mult)

            # ---- phase 5: MoE experts + shared + LN2 -----------------
            mv2 = apool.tile([P, TT, 2], f32)
            rstd2 = apool.tile([P, TT], f32)
            acc_pool = est.enter_context(tc.tile_pool(name="accs", bufs=4))
            out_pool = est.enter_context(tc.tile_pool(name="outs", bufs=3))

            def leaky_w_acc(out, psum, w_scalar, acc):
                if leaky_op is not None:
                    nc.vector._custom_dve(leaky_op, out=out, in0=psum,
                                          in1=acc, s0=w_scalar, s1=0.0,
                                          imm2=SLOPE)
                else:  # fallback: 3 standard DVE ops
                    tmp = acc_pool.tile([P, E], f32, tag="tmp")
                    nc.vector.tensor_scalar(out=tmp, in0=psum, scalar1=SLOPE,
                                            scalar2=None, op0=ALU.mult)
                    nc.vector.tensor_tensor(out=tmp, in0=tmp, in1=psum,
                                            op=ALU.max)
                    nc.vector.scalar_tensor_tensor(
                        out=out, in0=tmp, scalar=w_scalar, in1=acc,
                        op0=ALU.mult, op1=ALU.add)

            with tc.tile_pool(name="p5e", bufs=8, space="PSUM") as pe:
                for t in range(TT):
                    eps_list = []
                    for n in range(NE):
                        eps_list.append(
                            pe.tile([P, E], f32, tag="e", name=f"ep{t}_{n}"))
                    for k in range(KT):
                        lhs = x1T_sb[:, k, t * P:(t + 1) * P]
                        for n in range(NE):
                            nc.tensor.matmul(
                                eps_list[n], lhs, expT_sb[:, n, k, :],
                                start=(k == 0), stop=(k == KT - 1 and
                                                      not nonzero_bias))
                    if nonzero_bias:
                        for n in range(NE):
                            nc.tensor.matmul(eps_list[n],
                                             ones_row[:, t * P:(t + 1) * P],
                                             bias_sb[:, 6 + n, :],
                                             start=False, stop=True)
                    acc = acc_pool.tile([P, E], f32, tag="acc")
                    nc.vector.tensor_copy(out=acc, in_=x1_sb[:, t, :])
                    for n in range(NE):
                        nacc = acc_pool.tile([P, E], f32, tag="acc")
                        leaky_w_acc(nacc, eps_list[n], wts[:, t, n:n + 1],
                                    acc)
                        acc = nacc
                    # shared expert (combine weight 1.0)
                    sps = pe.tile([P, E], f32, tag="e")
                    for k in range(KT):
                        nc.tensor.matmul(
                            sps, x1T_sb[:, k, t * P:(t + 1) * P],
                            shT_sb[:, k, :],
                            start=(k == 0), stop=(k == KT - 1 and
                                                  not nonzero_bias))
                    if nonzero_bias:
                        nc.tensor.matmul(sps, ones_row[:, t * P:(t + 1) * P],
                                         bias_sb[:, 4, :], start=False,
                                         stop=True)
                    nacc = acc_pool.tile([P, E], f32, tag="acc")
                    leaky_w_acc(nacc, sps, 1.0, acc)
                    acc = nacc
                    st = stat_pool.tile([P, 6], f32, tag="bn")
                    nc.vector.bn_stats(out=st, in_=acc)
                    nc.vector.bn_aggr(out=mv2[:, t, :], in_=st)
                    # per-tile LN2 (sqrt table already resident after LN1)
                    std2 = stat_pool.tile([P, 1], f32, tag="std2")
                    nc.scalar.activation(out=std2, in_=mv2[:, t, 1:2],
                                         func=ACT.Sqrt, bias=eps_t)
                    nc.vector.reciprocal(rstd2[:, t:t + 1], std2)
                    xo = out_pool.tile([P, E], f32, tag="xo")
                    nc.vector.tensor_scalar(
                        out=xo, in0=acc,
                        scalar1=mv2[:, t, 0:1], scalar2=rstd2[:, t:t + 1],
                        op0=ALU.subtract, op1=ALU.mult)
                    if ln2_affine:
                        nc.vector.tensor_tensor(out=xo, in0=xo,
                                                in1=lnw_sb[:, 2, :],
                                                op=ALU.mult)
                        nc.vector.tensor_tensor(out=xo, in0=xo,
                                                in1=lnw_sb[:, 3, :],
                                                op=ALU.add)
                    nc.sync.dma_start(
                        out=x2_d.ap().rearrange(
                            "(t p) e -> p t e", p=P)[:, t, :],
                        in_=xo)

    nc.compile()
    return nc


def _get_program(nonzero_bias, ln1_affine, ln2_affine):
    key = (nonzero_bias, ln1_affine, ln2_affine)
    if key not in _CACHE:
        _CACHE[key] = _build(*key)
    return _CACHE[key]


def kernel(**inputs):
    from concourse.bass_utils import run_bass_kernel_spmd

    x = np.asarray(inputs["x"], np.float32)            # (L, B, E)
    wq, bq = np.asarray(inputs["wq"], np.float32), np.asarray(inputs["bq"], np.float32)
    wk, bk = np.asarray(inputs["wk"], np.float32), np.asarray(inputs["bk"], np.float32)
    wv, bv = np.asarray(inputs["wv"], np.float32), np.asarray(inputs["bv"], np.float32)
    in_w, in_b = np.asarray(inputs["in_w"], np.float32), np.asarray(inputs["in_b"], np.float32)
    out_w, out_b = np.asarray(inputs["out_w"], np.float32), np.asarray(inputs["out_b"], np.float32)
    ln1_g, ln1_b = np.asarray(inputs["ln1_g"], np.float32), np.asarray(inputs["ln1_b"], np.float32)
    ln2_g, ln2_b = np.asarray(inputs["ln2_g"], np.float32), np.asarray(inputs["ln2_b"], np.float32)
    sh_w, sh_b = np.asarray(inputs["sh_w"], np.float32), np.asarray(inputs["sh_b"], np.float32)
    gate_w, gate_b = np.asarray(inputs["gate_w"], np.float32), np.asarray(inputs["gate_b"], np.float32)
    exp_w, exp_b = np.asarray(inputs["exp_w"], np.float32), np.asarray(inputs["exp_b"], np.float32)

    wqi, wki, wvi = in_w[0:E], in_w[E:2 * E], in_w[2 * E:3 * E]
    bqi, bki, bvi = in_b[0:E], in_b[E:2 * E], in_b[2 * E:3 * E]
    Wq = wqi @ wq
    Wk = wki @ wk
    Wv = wvi @ wv
    bq_e = wqi @ bq + bqi
    bk_e = wki @ bk + bki
    bv_e = wvi @ bv + bvi

    nonzero_bias = bool(
        np.any(bq_e) or np.any(bk_e) or np.any(bv_e) or np.any(out_b)
        or np.any(sh_b) or np.any(gate_b) or np.any(exp_b))
    ln1_affine = bool(np.any(ln1_g != 1.0) or np.any(ln1_b))
    ln2_affine = bool(np.any(ln2_g != 1.0) or np.any(ln2_b))

    nc = _get_program(nonzero_bias, ln1_affine, ln2_affine)

    shared = {
        "wqT": np.ascontiguousarray(Wq.T).astype(bf16),
        "wkT": np.ascontiguousarray(Wk.T).astype(bf16),
        "wvT": np.ascontiguousarray(Wv.T).astype(bf16),
        "owT": np.ascontiguousarray(out_w.T).astype(bf16),
        "shT": np.ascontiguousarray(sh_w.T).astype(bf16),
        "expT": np.ascontiguousarray(np.transpose(exp_w, (0, 2, 1))).astype(bf16),
        "gwT": np.ascontiguousarray(gate_w.T).astype(bf16),
    }
    if nonzero_bias:
        biases = np.zeros((13, E), np.float32)
        biases[0], biases[1], biases[2] = bq_e, bk_e, bv_e
        biases[3], biases[4] = out_b, sh_b
        biases[5, :NE] = gate_b
        biases[6:6 + NE] = exp_b
        shared["biases"] = biases.astype(bf16)
    if ln1_affine or ln2_affine:
        lnw = np.stack([ln1_g, ln1_b, ln2_g, ln2_b]).astype(np.float32)
        shared["lnw"] = lnw

    in_maps = []
    for c in range(N_CORES):
        xb = np.ascontiguousarray(x[:, c, :])          # (L, E)
        m = dict(shared)
        m["xT"] = np.ascontiguousarray(xb.T).astype(bf16)
        m["xtok"] = xb
        in_maps.append(m)

    res = run_bass_kernel_spmd(nc, in_maps, core_ids=list(range(N_CORES)))

    x2 = np.stack([res.results[c]["x2"] for c in range(N_CORES)], axis=1)
    probs = np.stack([res.results[c]["probs"] for c in range(N_CORES)],
                     axis=1)                            # (L, B, NE) f32

    # ---- gating aux loss on host (needs global token means) ----------
    p = probs.reshape(-1, NE)
    m1 = p.max(axis=1, keepdims=True)
    mask1 = (p >= m1).astype(np.float32)
    pm = p - p * mask1
    m2 = pm.max(axis=1, keepdims=True)
    mask = (p >= m2).astype(np.float32)
    f = mask.mean(axis=0)
    Pm = p.mean(axis=0)
    gating_loss = np.float32(NE * np.sum(f * Pm))

    return x2.astype(np.float32), gating_loss
